# revision 11
# baseline (speedup 1.0000x reference)
"""Trainium2 Bass kernel for DVAE_INCI_WO_GNN forward (8 NeuronCores, SPMD).

Strategy (batch-parallel, 4 batches/core):
  - P = emb @ Wg computed on device with Wg column-sharded 8 ways (each core
    reads 1/8 of the two 10020x10020 matrices), then AllGather of the tiny
    [10, D] P matrices.  (hidden @ Wg == (emb @ Wg)[ops] since hidden rows
    are drawn from the 10-row embedding table.)
  - edges^T[b] = P^T @ M[b]^T with M[b] = onehot(ops[b])^T @ inci[b]  (rank-10).
  - inception: h1 = relu(A @ (x@W1)), h2 = relu(A^2 @ (x@W2)), A = inci^T inci.
    Main matmul (x @ [W0|W1|W2] for 4 inceptions) is batch-sharded; weights are
    read once per core as host-split bf16 hi/lo pairs and multiplied in a
    3-pass split-bf16 scheme (fp32-grade accuracy at 3/4 of fp32 PE cost).
  - LayerNorm over (rows, Z) slabs + uniform-noise reparam done per-core on
    [4, slab] layouts, output gathered on host.
"""

import os
from contextlib import ExitStack
import numpy as np
import ml_dtypes

import concourse.bass as bass
import concourse.tile as tile
from concourse import bacc, mybir
from concourse.masks import make_identity

F32 = mybir.dt.float32
BF16 = mybir.dt.bfloat16
AF = mybir.ActivationFunctionType

NCORES = 8
B, V, E, NT, Z = 32, 24, 48, 10, 56
BL = B // NCORES          # batches per core = 4
N = BL * E                # main matmul moving dim = 192
D = 10020
HID = 501


class Cfg:
    def __init__(self, d=D, hid=HID, name="full"):
        self.name = name
        self.d = d
        self.hid = hid
        self.dp = ((d + 1023) // 1024) * 1024      # D padded: shard*8, shard%128==0
        self.sh = self.dp // NCORES                # cols per core
        self.nch = self.sh // 128                  # P chunks per core
        self.kp = ((d + 127) // 128) * 128         # Wg row pad (P contraction)
        self.kt_p = self.kp // 128                 # P ktiles
        self.kt = self.dp // 128                   # main-matmul ktiles
        self.kb = 4                                # k blocks in main loop
        assert self.kt % self.kb == 0
        self.kk = self.kt // self.kb               # ktiles per block
        self.hidp = ((hid + 127) // 128) * 128     # padded W block width
        self.qt = self.hidp // 128                 # R Mtiles per W block
        self.wcols = 6 * self.hidp                 # Wcat cols per matrix
        self.mt = self.wcols // 128                # Wcat Mtiles per matrix
        self.hcat = 3 * self.hidp                  # padded concat width
        self.hkt = self.hcat // 128                # Wout ktiles
        self.se = Z * E                            # edge slab size = 2688
        self.sv = V * Z                            # vert slab size = 1344


FULL = Cfg()


def _bn_split(n):
    """free-dim subgroup size <= 512 dividing n."""
    for s in (512, 448, 384, 336, 256, 224, 192, 128, 112, 64, 56, 48, 32, 16, 8, 7, 4, 2, 1):
        if n % s == 0 and s <= 512:
            return s
    return 1


def build_nc(cfg=FULL):
    nc = bacc.Bacc("TRN2", target_bir_lowering=False, debug=False,
                   num_devices=NCORES)

    dI = lambda name, shape, dt=F32: nc.dram_tensor(name, shape, dt, kind="ExternalInput")
    # ---- parameters (per-core shards / replicas) ----
    embT = dI("embT", [cfg.kp, NT])                       # emb^T padded
    wg = {X: dI(f"wg_{X}", [cfg.kp, cfg.sh]) for X in "TH"}
    wc = {(X, h): dI(f"wcat_{X}_{h}", [cfg.dp, cfg.wcols], BF16)
          for X in "TH" for h in "hl"}
    wout = dI("wout", [4, cfg.hcat, Z])                   # muT,lvT,muH,lvH
    bout = dI("bout", [4, Z])
    wv = {j: dI(f"wv_{j}", [cfg.kp, Z]) for j in (0, 1)}  # mu, lv
    bv = dI("bv", [2, Z])
    inci = {X: dI(f"inci_{X}", [BL, V, E]) for X in "TH"}
    oneh = dI("onehot", [BL, V, NT])
    onehT = dI("onehotT", [BL, NT, V])
    ln_all = dI("ln_all", [12, cfg.se])                   # g/b packed rows
    noise = dI("noise", [12, cfg.se])                     # eT b0-3, eH b0-3, vert b0-3
    out = nc.dram_tensor("out", [BL, cfg.sv + 2 * cfg.se], F32, kind="ExternalOutput")

    # ---- internal DRAM ----
    ag_in = nc.dram_tensor("ag_in", [2 * NT, cfg.sh], F32)
    ag_out = nc.dram_tensor("ag_out", [NCORES * 2 * NT, cfg.sh], F32)
    zo = nc.dram_tensor("zo", [4, Z, N], F32)             # (X,i) out_incep^T scratch
    zv = nc.dram_tensor("zv", [2, BL, V, Z], F32)         # vert mu/lv scratch

    with tile.TileContext(nc) as tc:
        _emit(tc, cfg, embT, wg, wc, wout, bout, wv, bv, inci, oneh, onehT,
              ln_all, noise, out, ag_in, ag_out, zo, zv)
    nc.compile()
    return nc


def _emit(tc, cfg, embT, wg, wc, wout, bout, wv, bv, inci, oneh, onehT,
          ln_all, noise, out, ag_in, ag_out, zo, zv):
    nc = tc.nc
    TH = "TH"

    stack = ExitStack()
    const = stack.enter_context(tc.tile_pool(name="const", bufs=1))
    psmall = stack.enter_context(tc.tile_pool(name="psmall", bufs=2, space="PSUM"))

    # ---- constants into SBUF ----
    embT_sb = const.tile([128, cfg.kt_p, NT], F32)
    nc.sync.dma_start(embT_sb[:, :, :],
                      embT.ap().rearrange("(k p) t -> p k t", p=128))
    ident = const.tile([128, 128], F32)
    make_identity(nc, ident[:, :])
    inci_sb = {}
    for X in TH:
        inci_sb[X] = const.tile([V, BL, E], F32, tag=f"inci{X}", name=f"inci_sb{X}")
        nc.sync.dma_start(inci_sb[X][:, :, :],
                          inci[X].ap().rearrange("b v e -> v b e"))
    oneh_sb = const.tile([V, BL, NT], F32)
    nc.sync.dma_start(oneh_sb[:, :, :], oneh.ap().rearrange("b v t -> v b t"))
    onehT_sb = const.tile([NT, BL, V], F32)
    nc.sync.dma_start(onehT_sb[:, :, :], onehT.ap().rearrange("b t v -> t b v"))
    wout_sb = const.tile([128, 4, cfg.hkt, Z], F32)
    nc.sync.dma_start(wout_sb[:, :, :, :],
                      wout.ap().rearrange("w (k p) z -> p w k z", p=128))
    bout_sb = const.tile([Z, 4], F32)
    nc.sync.dma_start(bout_sb[:, :], bout.ap().rearrange("w z -> z w"))
    eps_sb = const.tile([BL, 1], F32)
    nc.vector.memset(eps_sb[:, :], 1e-5)

    # ---- A, A2, M (small PE work, runs during Wg streaming) ----
    A_sb, A2_sb, M_sb = {}, {}, {}
    for X in TH:
        A_sb[X] = const.tile([E, BL, E], F32, tag=f"A{X}", name=f"A_sb{X}")
        A2_sb[X] = const.tile([E, BL, E], F32, tag=f"A2{X}", name=f"A2_sb{X}")
        M_sb[X] = const.tile([NT, N], F32, tag=f"M{X}", name=f"M_sb{X}")
        for b in range(BL):
            pm = psmall.tile([NT, E], F32, tag="ps", name="pm")
            nc.tensor.matmul(pm[:, :],
                             lhsT=oneh_sb[:, b, :], rhs=inci_sb[X][:, b, :],
                             start=True, stop=True)
            nc.vector.tensor_copy(M_sb[X][:, E * b:E * (b + 1)], pm[:, :])
        for b in range(BL):
            pa = psmall.tile([E, E], F32, tag="ps")
            nc.tensor.matmul(pa[:, :], lhsT=inci_sb[X][:, b, :],
                             rhs=inci_sb[X][:, b, :], start=True, stop=True)
            nc.vector.tensor_copy(A_sb[X][:, b, :], pa[:, :])
            pa2 = psmall.tile([E, E], F32, tag="ps")
            nc.tensor.matmul(pa2[:, :], lhsT=A_sb[X][:, b, :],
                             rhs=A_sb[X][:, b, :], start=True, stop=True)
            nc.vector.tensor_copy(A2_sb[X][:, b, :], pa2[:, :])

    # ---- Phase P: P = emb @ Wg (column shard), AllGather ----
    with tc.tile_pool(name="wgp", bufs=3) as wgp, \
         tc.tile_pool(name="psum_p", bufs=1, space="PSUM") as psum_p:
        psP = {X: psum_p.tile([NT, cfg.sh], F32, tag=f"psP{X}", name=f"psP{X}") for X in TH}
        for k in range(cfg.kt_p):
            for X in TH:
                wgt = wgp.tile([128, cfg.sh], F32, tag="wg")
                nc.sync.dma_start(wgt[:, :], wg[X][128 * k:128 * (k + 1), :])
                for c0 in range(0, cfg.sh, 512):
                    c1 = min(c0 + 512, cfg.sh)
                    nc.tensor.matmul(psP[X][:, c0:c1],
                                     lhsT=embT_sb[:, k, :],
                                     rhs=wgt[:, c0:c1],
                                     start=(k == 0),
                                     stop=(k == cfg.kt_p - 1))
        for xi, X in enumerate(TH):
            stg = wgp.tile([NT, cfg.sh], F32, tag="stg")
            nc.vector.tensor_copy(stg[:, :], psP[X][:, :])
            nc.sync.dma_start(ag_in[NT * xi:NT * (xi + 1), :], stg[:, :])
    nc.gpsimd.collective_compute(
        "AllGather", mybir.AluOpType.bypass,
        replica_groups=[list(range(NCORES))],
        ins=[ag_in.ap().opt()], outs=[ag_out.ap().opt()])

    # ---- vert path: Pv = emb @ Wv, q = relu(Pv + bv), gather by ops ----
    with tc.tile_pool(name="wvp", bufs=3) as wvp:
        q_sb = {}
        for j in (0, 1):
            psq = psmall.tile([NT, Z], F32, tag="ps")
            for k in range(cfg.kt_p):
                wvt = wvp.tile([128, Z], F32, tag="wv")
                nc.sync.dma_start(wvt[:, :], wv[j][128 * k:128 * (k + 1), :])
                nc.tensor.matmul(psq[:, :], lhsT=embT_sb[:, k, :], rhs=wvt[:, :],
                                 start=(k == 0), stop=(k == cfg.kt_p - 1))
            bvb = wvp.tile([NT, Z], F32, tag="bvb")
            src = bv.ap()[j:j + 1, :]
            nc.sync.dma_start(bvb[:, :],
                              bass.AP(tensor=src.tensor, offset=src.offset,
                                      ap=[[0, NT], [1, Z]]))
            tq = wvp.tile([NT, Z], F32, tag="tq")
            nc.vector.tensor_add(tq[:, :], psq[:, :], bvb[:, :])
            q_sb[j] = const.tile([NT, Z], F32, tag=f"q{j}", name=f"q_sb{j}")
            nc.vector.tensor_scalar_max(q_sb[j][:, :], tq[:, :], 0.0)
            vq = wvp.tile([V, BL, Z], F32, tag="vq")
            for b in range(BL):
                pg = psmall.tile([V, Z], F32, tag="ps")
                nc.tensor.matmul(pg[:, :], lhsT=onehT_sb[:, b, :], rhs=q_sb[j][:, :],
                                 start=True, stop=True)
                nc.vector.tensor_copy(vq[:, b, :], pg[:, :])
            for b in range(BL):
                nc.sync.dma_start(zv.ap()[j, b, :, :], vq[:, b, :])

    # ---- main per-matrix pipeline ----
    with tc.tile_pool(name="plhs", bufs=4) as plhs_p, \
         tc.tile_pool(name="epool", bufs=1) as epool, \
         tc.tile_pool(name="etmp", bufs=2) as etmp, \
         tc.tile_pool(name="wcp", bufs=3) as wcp, \
         tc.tile_pool(name="rpool", bufs=1) as rpool, \
         tc.tile_pool(name="hpool", bufs=1) as hpool, \
         tc.tile_pool(name="xwp", bufs=4) as xwp, \
         tc.tile_pool(name="opool", bufs=2) as opool, \
         tc.tile_pool(name="slab", bufs=1) as slab_p, \
         tc.tile_pool(name="bcast", bufs=1) as bc_p, \
         tc.tile_pool(name="psum_e", bufs=2, space="PSUM") as psum_e, \
         tc.tile_pool(name="psum_m", bufs=2, space="PSUM") as psum_m:

        for xi, X in enumerate(TH):
            # -- edges: e^T [dp, N] in bf16 hi/lo --
            e_hi = epool.tile([128, cfg.kt, N], BF16, tag="ehi")
            e_lo = epool.tile([128, cfg.kt, N], BF16, tag="elo")
            for k in range(cfg.kt):
                r, kk = divmod(k, cfg.nch)
                pl = plhs_p.tile([NT, 128], F32, tag="pl")
                nc.sync.dma_start(
                    pl[:, :],
                    ag_out.ap()[2 * NT * r + NT * xi: 2 * NT * r + NT * (xi + 1),
                                128 * kk:128 * (kk + 1)])
                pe = psum_e.tile([128, N], F32, tag="pe")
                nc.tensor.matmul(pe[:, :], lhsT=pl[:, :], rhs=M_sb[X][:, :],
                                 start=True, stop=True)
                tmp = etmp.tile([128, N], F32, tag="etmp")
                nc.scalar.activation(tmp[:, :], pe[:, :], AF.Relu)
                nc.scalar.activation(e_hi[:, k, :], pe[:, :], AF.Relu)
                hi32 = etmp.tile([128, N], F32, tag="hi32")
                nc.vector.tensor_copy(hi32[:, :], e_hi[:, k, :])
                diff = etmp.tile([128, N], F32, tag="diff")
                nc.vector.tensor_sub(diff[:, :], tmp[:, :], hi32[:, :])
                nc.vector.tensor_copy(e_lo[:, k, :], diff[:, :])

            # -- main matmul: R = (x @ Wcat)^T, 3-pass split bf16 --
            R_sb = rpool.tile([128, cfg.mt, N], F32, tag="R")
            for m in range(cfg.mt):
                pr = psum_m.tile([128, N], F32, tag="pr")
                for kb in range(cfg.kb):
                    sl = {}
                    for h in "hl":
                        wt = wcp.tile([128, cfg.kk, 128], BF16, tag=f"wc{h}")
                        nc.sync.dma_start(
                            wt[:, :, :],
                            wc[(X, h)].ap().rearrange(
                                "(kb kk p) (m c) -> kb m p kk c",
                                kk=cfg.kk, p=128, c=128)[kb, m])
                        sl[h] = wt
                    for kk in range(cfg.kk):
                        k = cfg.kk * kb + kk
                        last = (k == cfg.kt - 1)
                        nc.tensor.matmul(pr[:, :], lhsT=sl["h"][:, kk, :],
                                         rhs=e_hi[:, k, :],
                                         start=(k == 0), stop=False)
                        nc.tensor.matmul(pr[:, :], lhsT=sl["h"][:, kk, :],
                                         rhs=e_lo[:, k, :],
                                         start=False, stop=False)
                        nc.tensor.matmul(pr[:, :], lhsT=sl["l"][:, kk, :],
                                         rhs=e_hi[:, k, :],
                                         start=False, stop=last)
                nc.vector.tensor_copy(R_sb[:, m, :], pr[:, :])

            # -- final per inception i: hcat^T -> Wout -> zo --
            for i in range(2):
                w_idx = 2 * xi + i
                hcat = hpool.tile([128, cfg.hkt, N], F32, tag="hcat")
                for q in range(cfg.qt):
                    nc.vector.tensor_scalar_max(
                        hcat[:, q, :], R_sb[:, cfg.qt * (3 * i) + q, :], 0.0)
                for wnum, amat in ((1, A_sb[X]), (2, A2_sb[X])):
                    for q in range(cfg.qt):
                        for b in range(BL):
                            pt = psum_e.tile([E, 128], F32, tag="pe", name="pt")
                            nc.tensor.transpose(
                                pt[:, :],
                                R_sb[:, cfg.qt * (3 * i + wnum) + q,
                                     E * b:E * (b + 1)],
                                ident[:, :])
                            xw = xwp.tile([E, 128], F32, tag="xw", name="xw")
                            nc.vector.tensor_copy(xw[:, :], pt[:, :])
                            ph = psum_m.tile([128, E], F32, tag="pr", name="ph")
                            nc.tensor.matmul(ph[:, :],
                                             lhsT=xw[:, :], rhs=amat[:, b, :],
                                             start=True, stop=True)
                            nc.vector.tensor_scalar_max(
                                hcat[:, cfg.qt * wnum + q, E * b:E * (b + 1)],
                                ph[:, :], 0.0)
                po = psum_m.tile([Z, N], F32, tag="pr")
                for k2 in range(cfg.hkt):
                    nc.tensor.matmul(po[:, :], lhsT=wout_sb[:, w_idx, k2, :],
                                     rhs=hcat[:, k2, :],
                                     start=(k2 == 0), stop=(k2 == cfg.hkt - 1))
                ot = opool.tile([Z, N], F32, tag="ot")
                nc.vector.tensor_scalar(ot[:, :], po[:, :],
                                        bout_sb[:, w_idx:w_idx + 1], None,
                                        op0=mybir.AluOpType.add)
                nc.sync.dma_start(zo.ap()[w_idx], ot[:, :])

        # ---- LN + reparam ----
        def layer_norm_reparam(slab_mu, slab_lv, nsz, g_rows, noise_row, out_off):
            """slab_*: [BL, nsz] tiles (modified in place)."""
            sub = _bn_split(nsz)
            g_mu, b_mu, g_lv, b_lv = g_rows

            def bcast(row):
                t = bc_p.tile([BL, nsz], F32, tag="bc", name="bc")
                src = ln_all.ap()[row:row + 1, 0:nsz]
                nc.sync.dma_start(
                    t[:, :], bass.AP(tensor=src.tensor, offset=src.offset,
                                     ap=[[0, BL], [1, nsz]]))
                return t

            for sl, g_row, b_row in ((slab_mu, g_mu, b_mu), (slab_lv, g_lv, b_lv)):
                st = slab_p.tile([BL, nsz // sub, 6], F32, tag="st", name="st")
                slview = sl.rearrange("p (a b) -> p a b", b=sub)
                for a in range(nsz // sub):
                    nc.vector.bn_stats(st[:, a, :], slview[:, a, :])
                mv2 = slab_p.tile([BL, 2], F32, tag="mv2", name="mv2")
                nc.vector.bn_aggr(mv2[:, :], st[:, :, :])
                sq = slab_p.tile([BL, 1], F32, tag="sq", name="sq")
                nc.scalar.activation(sq[:, :], mv2[:, 1:2], AF.Sqrt,
                                     bias=eps_sb[:, :])
                rstd = slab_p.tile([BL, 1], F32, tag="rstd", name="rstd")
                nc.vector.reciprocal(rstd[:, :], sq[:, :])
                # in-place: sl = ((sl - mean) * rstd) * g + b
                nc.vector.tensor_scalar(sl[:, :], sl[:, :], mv2[:, 0:1],
                                        rstd[:, :],
                                        op0=mybir.AluOpType.subtract,
                                        op1=mybir.AluOpType.mult)
                nc.vector.tensor_mul(sl[:, :], sl[:, :], bcast(g_row)[:, :])
                nc.vector.tensor_add(sl[:, :], sl[:, :], bcast(b_row)[:, :])
            # slab_lv = exp(0.5 * slab_lv); z = noise * that + slab_mu
            nc.scalar.activation(slab_lv[:, :], slab_lv[:, :], AF.Exp, scale=0.5)
            nz = slab_p.tile([BL, nsz], F32, tag="nz", name="nz")
            nc.sync.dma_start(nz[:, :], noise.ap()[noise_row:noise_row + BL, 0:nsz])
            nc.vector.tensor_mul(nz[:, :], nz[:, :], slab_lv[:, :])
            nc.vector.tensor_add(nz[:, :], nz[:, :], slab_mu[:, :])
            nc.sync.dma_start(out.ap()[:, out_off:out_off + nsz], nz[:, :])

        # vert: slabs [BL, V*Z] natural (v,z) order
        sv_mu = slab_p.tile([BL, cfg.se], F32, tag="smu", name="sv_mu")
        nc.sync.dma_start(sv_mu[:, 0:cfg.sv],
                          zv.ap().rearrange("j b v z -> j b (v z)")[0])
        sv_lv = slab_p.tile([BL, cfg.se], F32, tag="slv", name="sv_lv")
        nc.sync.dma_start(sv_lv[:, 0:cfg.sv],
                          zv.ap().rearrange("j b v z -> j b (v z)")[1])
        layer_norm_reparam(sv_mu[:, 0:cfg.sv], sv_lv[:, 0:cfg.sv], cfg.sv,
                           (8, 9, 10, 11), 8, 0)

        # edges: slabs [BL, Z*E] in (z,e) order from zo [w, Z, N]
        for gi, (w_mu, w_lv) in enumerate(((0, 1), (2, 3))):
            smu = slab_p.tile([BL, cfg.se], F32, tag="smu", name="smu")
            slv = slab_p.tile([BL, cfg.se], F32, tag="slv", name="slv")
            for w, sl in ((w_mu, smu), (w_lv, slv)):
                src = zo.ap()[w]  # [Z, N]; slab[b,(z,e)] = zo[w,z,48b+e]
                nc.sync.dma_start(
                    sl[:, :], bass.AP(tensor=src.tensor, offset=src.offset,
                                      ap=[[E, BL], [N, Z], [1, E]]))
            layer_norm_reparam(smu, slv, cfg.se, tuple(4 * gi + j for j in range(4)),
                               4 * gi, cfg.sv + cfg.se * gi)

    stack.close()


# ---------------- host side ----------------

def _pad(a, shape):
    r = np.zeros(shape, dtype=np.float32)
    r[tuple(slice(0, s) for s in a.shape)] = a
    return r


def prep_inputs(ops, inci_T, inci_H, noise_vert, noise_eT, noise_eH, params,
                cfg=FULL):
    p = params
    emb = np.asarray(p["emb"], np.float32)
    in_maps = []
    embT_h = _pad(emb.T, (cfg.kp, NT))
    wout_h = np.zeros((4, cfg.hcat, Z), np.float32)
    wcat = {}
    for w_idx, nm in enumerate(("muT", "lvT", "muH", "lvH")):
        pp = p[nm]
        for j, wn in enumerate(("W0", "W1", "W2")):
            wout_h[w_idx, cfg.hidp * j:cfg.hidp * j + cfg.hid, :] = \
                np.asarray(pp["Wout"], np.float32)[cfg.hid * j:cfg.hid * (j + 1), :]
    for X, names in (("T", ("muT", "lvT")), ("H", ("muH", "lvH"))):
        cat = np.zeros((cfg.dp, cfg.wcols), np.float32)
        for i, nm in enumerate(names):
            pp = p[nm]
            for j, wn in enumerate(("W0", "W1", "W2")):
                c0 = cfg.hidp * (3 * i + j)
                cat[:cfg.d, c0:c0 + cfg.hid] = np.asarray(pp[wn], np.float32)
        hi = cat.astype(ml_dtypes.bfloat16)
        lo = (cat - hi.astype(np.float32)).astype(ml_dtypes.bfloat16)
        wcat[(X, "h")], wcat[(X, "l")] = hi, lo
    bout_h = np.stack([np.asarray(p[nm]["bout"], np.float32)
                       for nm in ("muT", "lvT", "muH", "lvH")])
    wg_pad = {X: _pad(np.asarray(p[f"Wg_{X}"], np.float32), (cfg.kp, cfg.dp))
              for X in "TH"}
    wv_h = {j: _pad(np.asarray(p[nm], np.float32), (cfg.kp, Z))
            for j, nm in ((0, "Wv_mu"), (1, "Wv_lv"))}
    bv_h = np.stack([np.asarray(p["bv_mu"], np.float32),
                     np.asarray(p["bv_lv"], np.float32)])
    ln = p["ln"]
    ln_h = np.zeros((12, cfg.se), np.float32)
    for gi, (gm, gl) in enumerate((("meT", "leT"), ("meH", "leH"))):
        ln_h[4 * gi + 0] = np.asarray(ln[gm]["g"], np.float32).T.ravel()
        ln_h[4 * gi + 1] = np.asarray(ln[gm]["b"], np.float32).T.ravel()
        ln_h[4 * gi + 2] = np.asarray(ln[gl]["g"], np.float32).T.ravel()
        ln_h[4 * gi + 3] = np.asarray(ln[gl]["b"], np.float32).T.ravel()
    ln_h[8, :cfg.sv] = np.asarray(ln["mv"]["g"], np.float32).ravel()
    ln_h[9, :cfg.sv] = np.asarray(ln["mv"]["b"], np.float32).ravel()
    ln_h[10, :cfg.sv] = np.asarray(ln["lv"]["g"], np.float32).ravel()
    ln_h[11, :cfg.sv] = np.asarray(ln["lv"]["b"], np.float32).ravel()

    ops_n = np.asarray(ops)
    onehot = np.eye(NT, dtype=np.float32)[ops_n]          # [B, V, NT]
    for c in range(NCORES):
        bs = slice(BL * c, BL * (c + 1))
        noise_h = np.zeros((12, cfg.se), np.float32)
        noise_h[0:4] = np.asarray(noise_eT, np.float32)[bs].transpose(0, 2, 1).reshape(BL, cfg.se)
        noise_h[4:8] = np.asarray(noise_eH, np.float32)[bs].transpose(0, 2, 1).reshape(BL, cfg.se)
        noise_h[8:12, :cfg.sv] = np.asarray(noise_vert, np.float32)[bs].reshape(BL, cfg.sv)
        m = {
            "embT": embT_h,
            "wg_T": wg_pad["T"][:, cfg.sh * c:cfg.sh * (c + 1)].copy(),
            "wg_H": wg_pad["H"][:, cfg.sh * c:cfg.sh * (c + 1)].copy(),
            "wcat_T_h": wcat[("T", "h")], "wcat_T_l": wcat[("T", "l")],
            "wcat_H_h": wcat[("H", "h")], "wcat_H_l": wcat[("H", "l")],
            "wout": wout_h, "bout": bout_h,
            "wv_0": wv_h[0], "wv_1": wv_h[1], "bv": bv_h,
            "inci_T": np.ascontiguousarray(np.asarray(inci_T, np.float32)[bs]),
            "inci_H": np.ascontiguousarray(np.asarray(inci_H, np.float32)[bs]),
            "onehot": np.ascontiguousarray(onehot[bs]),
            "onehotT": np.ascontiguousarray(onehot[bs].transpose(0, 2, 1)),
            "ln_all": ln_h, "noise": noise_h,
        }
        in_maps.append(m)
    return in_maps


def assemble_output(results, cfg=FULL):
    outs = []
    for c in range(NCORES):
        o = np.asarray(results[c]["out"], np.float32)     # [BL, sv+2*se]
        z_v = o[:, :cfg.sv].reshape(BL, V, Z)
        z_T = o[:, cfg.sv:cfg.sv + cfg.se].reshape(BL, Z, E).transpose(0, 2, 1)
        z_H = o[:, cfg.sv + cfg.se:].reshape(BL, Z, E).transpose(0, 2, 1)
        outs.append(np.concatenate([z_v, z_T, z_H], axis=1))
    return np.concatenate(outs, axis=0)


_STATE = {}


def _ensure_ntff_hook():
    """Inject antenv.axon_hooks (missing in this image) so trace=True works."""
    import sys, types
    try:
        import antenv.axon_hooks  # noqa: F401
        return
    except ImportError:
        pass
    try:
        from trn_agent_boot.trn_boot import _ntff_profile_via_ctypes
        hook = _ntff_profile_via_ctypes("/opt/axon/libaxon_pjrt.so")
        mod = types.ModuleType("antenv.axon_hooks")
        mod._hook = hook
        mod.set_axon_ntff_profile_hook = lambda h: setattr(mod, "_hook", h)
        mod.get_axon_ntff_profile_hook = lambda: mod._hook
        sys.modules["antenv.axon_hooks"] = mod
        import antenv
        antenv.axon_hooks = mod
    except Exception:
        pass


def kernel(ops, inci_T, inci_H, noise_vert, noise_eT, noise_eH, params):
    from concourse.bass_utils import run_bass_kernel_spmd
    if bool(int(os.environ.get("BASS_KERNEL_TRACE", "0"))):
        _ensure_ntff_hook()
    if "nc" not in _STATE:
        _STATE["nc"] = build_nc(FULL)
    nc = _STATE["nc"]
    in_maps = prep_inputs(ops, inci_T, inci_H, noise_vert, noise_eT, noise_eH,
                          params, FULL)
    trace = bool(int(os.environ.get("BASS_KERNEL_TRACE", "0")))
    res = run_bass_kernel_spmd(nc, in_maps, core_ids=list(range(NCORES)),
                               trace=trace)
    _STATE["last_result"] = res
    return assemble_output(res.results, FULL)


# revision 12
# speedup vs baseline: 1.0347x; 1.0347x over previous
"""Trainium2 Bass kernel for DVAE_INCI_WO_GNN forward (8 NeuronCores, SPMD).

Strategy (batch-parallel, 4 batches/core):
  - P = emb @ Wg computed on device with Wg column-sharded 8 ways (each core
    reads 1/8 of the two 10020x10020 matrices), then AllGather of the tiny
    [10, D] P matrices.  (hidden @ Wg == (emb @ Wg)[ops] since hidden rows
    are drawn from the 10-row embedding table.)
  - edges^T[b] = P^T @ M[b]^T with M[b] = onehot(ops[b])^T @ inci[b]  (rank-10).
  - inception: h1 = relu(A @ (x@W1)), h2 = relu(A^2 @ (x@W2)), A = inci^T inci.
    Main matmul (x @ [W0|W1|W2] for 4 inceptions) is batch-sharded; weights are
    read once per core as host-split bf16 hi/lo pairs and multiplied in a
    3-pass split-bf16 scheme (fp32-grade accuracy at 3/4 of fp32 PE cost).
  - LayerNorm over (rows, Z) slabs + uniform-noise reparam done per-core on
    [4, slab] layouts, output gathered on host.
"""

import os
from contextlib import ExitStack
import numpy as np
import ml_dtypes

import concourse.bass as bass
import concourse.tile as tile
from concourse import bacc, mybir
from concourse.masks import make_identity

F32 = mybir.dt.float32
BF16 = mybir.dt.bfloat16
AF = mybir.ActivationFunctionType

NCORES = 8
B, V, E, NT, Z = 32, 24, 48, 10, 56
BL = B // NCORES          # batches per core = 4
N = BL * E                # main matmul moving dim = 192
D = 10020
HID = 501


class Cfg:
    def __init__(self, d=D, hid=HID, name="full"):
        self.name = name
        self.d = d
        self.hid = hid
        self.dp = ((d + 1023) // 1024) * 1024      # D padded: shard*8, shard%128==0
        self.sh = self.dp // NCORES                # cols per core
        self.nch = self.sh // 128                  # P chunks per core
        self.kp = ((d + 127) // 128) * 128         # Wg row pad (P contraction)
        self.kt_p = self.kp // 128                 # P ktiles
        self.kt = self.dp // 128                   # main-matmul ktiles
        self.kb = 4                                # k blocks in main loop
        assert self.kt % self.kb == 0
        self.kk = self.kt // self.kb               # ktiles per block
        self.hidp = ((hid + 127) // 128) * 128     # padded W block width
        self.qt = self.hidp // 128                 # R Mtiles per W block
        self.wcols = 6 * self.hidp                 # Wcat cols per matrix
        self.mt = self.wcols // 128                # Wcat Mtiles per matrix
        self.hcat = 3 * self.hidp                  # padded concat width
        self.hkt = self.hcat // 128                # Wout ktiles
        self.se = Z * E                            # edge slab size = 2688
        self.sv = V * Z                            # vert slab size = 1344


FULL = Cfg()


def _bn_split(n):
    """free-dim subgroup size <= 512 dividing n."""
    for s in (512, 448, 384, 336, 256, 224, 192, 128, 112, 64, 56, 48, 32, 16, 8, 7, 4, 2, 1):
        if n % s == 0 and s <= 512:
            return s
    return 1


def build_nc(cfg=FULL):
    nc = bacc.Bacc("TRN2", target_bir_lowering=False, debug=False,
                   num_devices=NCORES)

    dI = lambda name, shape, dt=F32: nc.dram_tensor(name, shape, dt, kind="ExternalInput")
    # ---- parameters (per-core shards / replicas) ----
    embT = dI("embT", [cfg.kp, NT])                       # emb^T padded
    wg = {(X, h): dI(f"wg_{X}_{h}", [cfg.kp, cfg.sh], BF16)
          for X in "TH" for h in "hl"}
    embTb = {h: dI(f"embTb_{h}", [cfg.kp, NT], BF16) for h in "hl"}
    wc = {(X, h): dI(f"wcat_{X}_{h}", [cfg.mt, cfg.kb, 128, cfg.kk * 128], BF16)
          for X in "TH" for h in "hl"}
    wout = dI("wout", [4, cfg.hcat, Z])                   # muT,lvT,muH,lvH
    bout = dI("bout", [4, Z])
    wv = {j: dI(f"wv_{j}", [cfg.kp, Z]) for j in (0, 1)}  # mu, lv
    bv = dI("bv", [2, Z])
    inci = {X: dI(f"inci_{X}", [BL, V, E]) for X in "TH"}
    oneh = dI("onehot", [BL, V, NT])
    onehT = dI("onehotT", [BL, NT, V])
    ln_all = dI("ln_all", [12, cfg.se])                   # g/b packed rows
    noise = dI("noise", [12, cfg.se])                     # eT b0-3, eH b0-3, vert b0-3
    out = nc.dram_tensor("out", [BL, cfg.sv + 2 * cfg.se], F32, kind="ExternalOutput")

    # ---- internal DRAM ----
    ag_in = nc.dram_tensor("ag_in", [2 * NT, cfg.sh], F32)
    ag_out = nc.dram_tensor("ag_out", [NCORES * 2 * NT, cfg.sh], F32)
    zo = nc.dram_tensor("zo", [4, Z, N], F32)             # (X,i) out_incep^T scratch
    zv = nc.dram_tensor("zv", [2, BL, V, Z], F32)         # vert mu/lv scratch

    with tile.TileContext(nc) as tc:
        _emit(tc, cfg, embT, embTb, wg, wc, wout, bout, wv, bv, inci, oneh, onehT,
              ln_all, noise, out, ag_in, ag_out, zo, zv)
    nc.compile()
    return nc


def _emit(tc, cfg, embT, embTb, wg, wc, wout, bout, wv, bv, inci, oneh, onehT,
          ln_all, noise, out, ag_in, ag_out, zo, zv):
    nc = tc.nc
    TH = "TH"

    stack = ExitStack()
    const = stack.enter_context(tc.tile_pool(name="const", bufs=1))
    psmall = stack.enter_context(tc.tile_pool(name="psmall", bufs=2, space="PSUM"))

    # ---- constants into SBUF ----
    embT_sb = const.tile([128, cfg.kt_p, NT], F32)
    nc.sync.dma_start(embT_sb[:, :, :],
                      embT.ap().rearrange("(k p) t -> p k t", p=128))
    embTb_sb = {}
    for h in "hl":
        embTb_sb[h] = const.tile([128, cfg.kt_p, NT], BF16, tag=f"embTb{h}",
                                 name=f"embTb_sb{h}")
        nc.sync.dma_start(embTb_sb[h][:, :, :],
                          embTb[h].ap().rearrange("(k p) t -> p k t", p=128))
    ident = const.tile([128, 128], F32)
    make_identity(nc, ident[:, :])
    inci_sb = {}
    for X in TH:
        inci_sb[X] = const.tile([V, BL, E], F32, tag=f"inci{X}", name=f"inci_sb{X}")
        nc.sync.dma_start(inci_sb[X][:, :, :],
                          inci[X].ap().rearrange("b v e -> v b e"))
    oneh_sb = const.tile([V, BL, NT], F32)
    nc.sync.dma_start(oneh_sb[:, :, :], oneh.ap().rearrange("b v t -> v b t"))
    onehT_sb = const.tile([NT, BL, V], F32)
    nc.sync.dma_start(onehT_sb[:, :, :], onehT.ap().rearrange("b t v -> t b v"))
    wout_sb = const.tile([128, 4, cfg.hkt, Z], F32)
    nc.sync.dma_start(wout_sb[:, :, :, :],
                      wout.ap().rearrange("w (k p) z -> p w k z", p=128))
    bout_sb = const.tile([Z, 4], F32)
    nc.sync.dma_start(bout_sb[:, :], bout.ap().rearrange("w z -> z w"))
    eps_sb = const.tile([BL, 1], F32)
    nc.vector.memset(eps_sb[:, :], 1e-5)

    # ---- A, A2, M (small PE work, runs during Wg streaming) ----
    A_sb, A2_sb, M_sb = {}, {}, {}
    for X in TH:
        A_sb[X] = const.tile([E, BL, E], F32, tag=f"A{X}", name=f"A_sb{X}")
        A2_sb[X] = const.tile([E, BL, E], F32, tag=f"A2{X}", name=f"A2_sb{X}")
        M_sb[X] = const.tile([NT, N], F32, tag=f"M{X}", name=f"M_sb{X}")
        for b in range(BL):
            pm = psmall.tile([NT, E], F32, tag="ps", name="pm")
            nc.tensor.matmul(pm[:, :],
                             lhsT=oneh_sb[:, b, :], rhs=inci_sb[X][:, b, :],
                             start=True, stop=True)
            nc.vector.tensor_copy(M_sb[X][:, E * b:E * (b + 1)], pm[:, :])
        for b in range(BL):
            pa = psmall.tile([E, E], F32, tag="ps")
            nc.tensor.matmul(pa[:, :], lhsT=inci_sb[X][:, b, :],
                             rhs=inci_sb[X][:, b, :], start=True, stop=True)
            nc.vector.tensor_copy(A_sb[X][:, b, :], pa[:, :])
            pa2 = psmall.tile([E, E], F32, tag="ps")
            nc.tensor.matmul(pa2[:, :], lhsT=A_sb[X][:, b, :],
                             rhs=A_sb[X][:, b, :], start=True, stop=True)
            nc.vector.tensor_copy(A2_sb[X][:, b, :], pa2[:, :])

    # ---- Phase P: P = emb @ Wg (column shard), AllGather ----
    with tc.tile_pool(name="wgp", bufs=3) as wgp, \
         tc.tile_pool(name="psum_p", bufs=1, space="PSUM") as psum_p:
        psP = {X: psum_p.tile([NT, cfg.sh], F32, tag=f"psP{X}", name=f"psP{X}") for X in TH}
        for k in range(cfg.kt_p):
            for X in TH:
                wgh = wgp.tile([128, cfg.sh], BF16, tag="wgh", name="wgh")
                nc.sync.dma_start(wgh[:, :], wg[(X, "h")][128 * k:128 * (k + 1), :])
                wgl = wgp.tile([128, cfg.sh], BF16, tag="wgl", name="wgl")
                nc.sync.dma_start(wgl[:, :], wg[(X, "l")][128 * k:128 * (k + 1), :])
                passes = ((embTb_sb["h"], wgh), (embTb_sb["h"], wgl),
                          (embTb_sb["l"], wgh))
                for a, (lh, rh) in enumerate(passes):
                    for c0 in range(0, cfg.sh, 512):
                        c1 = min(c0 + 512, cfg.sh)
                        nc.tensor.matmul(psP[X][:, c0:c1],
                                         lhsT=lh[:, k, :],
                                         rhs=rh[:, c0:c1],
                                         start=(k == 0 and a == 0),
                                         stop=(k == cfg.kt_p - 1 and a == 2))
        for xi, X in enumerate(TH):
            stg = wgp.tile([NT, cfg.sh], F32, tag="stg")
            nc.vector.tensor_copy(stg[:, :], psP[X][:, :])
            nc.sync.dma_start(ag_in[NT * xi:NT * (xi + 1), :], stg[:, :])
    nc.gpsimd.collective_compute(
        "AllGather", mybir.AluOpType.bypass,
        replica_groups=[list(range(NCORES))],
        ins=[ag_in.ap().opt()], outs=[ag_out.ap().opt()])

    # ---- vert path: Pv = emb @ Wv, q = relu(Pv + bv), gather by ops ----
    with tc.tile_pool(name="wvp", bufs=3) as wvp:
        q_sb = {}
        for j in (0, 1):
            psq = psmall.tile([NT, Z], F32, tag="ps")
            for k in range(cfg.kt_p):
                wvt = wvp.tile([128, Z], F32, tag="wv")
                nc.sync.dma_start(wvt[:, :], wv[j][128 * k:128 * (k + 1), :])
                nc.tensor.matmul(psq[:, :], lhsT=embT_sb[:, k, :], rhs=wvt[:, :],
                                 start=(k == 0), stop=(k == cfg.kt_p - 1))
            bvb = wvp.tile([NT, Z], F32, tag="bvb")
            src = bv.ap()[j:j + 1, :]
            nc.sync.dma_start(bvb[:, :],
                              bass.AP(tensor=src.tensor, offset=src.offset,
                                      ap=[[0, NT], [1, Z]]))
            tq = wvp.tile([NT, Z], F32, tag="tq")
            nc.vector.tensor_add(tq[:, :], psq[:, :], bvb[:, :])
            q_sb[j] = const.tile([NT, Z], F32, tag=f"q{j}", name=f"q_sb{j}")
            nc.vector.tensor_scalar_max(q_sb[j][:, :], tq[:, :], 0.0)
            vq = wvp.tile([V, BL, Z], F32, tag="vq")
            for b in range(BL):
                pg = psmall.tile([V, Z], F32, tag="ps")
                nc.tensor.matmul(pg[:, :], lhsT=onehT_sb[:, b, :], rhs=q_sb[j][:, :],
                                 start=True, stop=True)
                nc.vector.tensor_copy(vq[:, b, :], pg[:, :])
            for b in range(BL):
                nc.sync.dma_start(zv.ap()[j, b, :, :], vq[:, b, :])

    # ---- main per-matrix pipeline ----
    with tc.tile_pool(name="plhs", bufs=4) as plhs_p, \
         tc.tile_pool(name="epool", bufs=1) as epool, \
         tc.tile_pool(name="etmp", bufs=2) as etmp, \
         tc.tile_pool(name="wcp", bufs=3) as wcp, \
         tc.tile_pool(name="rpool", bufs=1) as rpool, \
         tc.tile_pool(name="hpool", bufs=1) as hpool, \
         tc.tile_pool(name="xwp", bufs=4) as xwp, \
         tc.tile_pool(name="opool", bufs=2) as opool, \
         tc.tile_pool(name="slab", bufs=1) as slab_p, \
         tc.tile_pool(name="bcast", bufs=1) as bc_p, \
         tc.tile_pool(name="psum_e", bufs=2, space="PSUM") as psum_e, \
         tc.tile_pool(name="psum_m", bufs=2, space="PSUM") as psum_m:

        for xi, X in enumerate(TH):
            # -- edges: e^T [dp, N] in bf16 hi/lo --
            e_hi = epool.tile([128, cfg.kt, N], BF16, tag="ehi")
            e_lo = epool.tile([128, cfg.kt, N], BF16, tag="elo")
            for k in range(cfg.kt):
                r, kk = divmod(k, cfg.nch)
                pl = plhs_p.tile([NT, 128], F32, tag="pl")
                nc.sync.dma_start(
                    pl[:, :],
                    ag_out.ap()[2 * NT * r + NT * xi: 2 * NT * r + NT * (xi + 1),
                                128 * kk:128 * (kk + 1)])
                pe = psum_e.tile([128, N], F32, tag="pe")
                nc.tensor.matmul(pe[:, :], lhsT=pl[:, :], rhs=M_sb[X][:, :],
                                 start=True, stop=True)
                tmp = etmp.tile([128, N], F32, tag="etmp")
                nc.scalar.activation(tmp[:, :], pe[:, :], AF.Relu)
                nc.scalar.activation(e_hi[:, k, :], pe[:, :], AF.Relu)
                hi32 = etmp.tile([128, N], F32, tag="hi32")
                nc.vector.tensor_copy(hi32[:, :], e_hi[:, k, :])
                diff = etmp.tile([128, N], F32, tag="diff")
                nc.vector.tensor_sub(diff[:, :], tmp[:, :], hi32[:, :])
                nc.vector.tensor_copy(e_lo[:, k, :], diff[:, :])

            # -- main matmul: R = (x @ Wcat)^T, 3-pass split bf16 --
            R_sb = rpool.tile([128, cfg.mt, N], F32, tag="R")
            for m in range(cfg.mt):
                pr = psum_m.tile([128, N], F32, tag="pr")
                for kb in range(cfg.kb):
                    sl = {}
                    for h in "hl":
                        wt = wcp.tile([128, cfg.kk, 128], BF16, tag=f"wc{h}")
                        nc.sync.dma_start(
                            wt[:, :, :],
                            wc[(X, h)].ap()[m, kb].rearrange(
                                "p (kk c) -> p kk c", c=128))
                        sl[h] = wt
                    for kk in range(cfg.kk):
                        k = cfg.kk * kb + kk
                        last = (k == cfg.kt - 1)
                        nc.tensor.matmul(pr[:, :], lhsT=sl["h"][:, kk, :],
                                         rhs=e_hi[:, k, :],
                                         start=(k == 0), stop=False)
                        nc.tensor.matmul(pr[:, :], lhsT=sl["h"][:, kk, :],
                                         rhs=e_lo[:, k, :],
                                         start=False, stop=False)
                        nc.tensor.matmul(pr[:, :], lhsT=sl["l"][:, kk, :],
                                         rhs=e_hi[:, k, :],
                                         start=False, stop=last)
                nc.vector.tensor_copy(R_sb[:, m, :], pr[:, :])

            # -- final per inception i: hcat^T -> Wout -> zo --
            for i in range(2):
                w_idx = 2 * xi + i
                hcat = hpool.tile([128, cfg.hkt, N], F32, tag="hcat")
                for q in range(cfg.qt):
                    nc.vector.tensor_scalar_max(
                        hcat[:, q, :], R_sb[:, cfg.qt * (3 * i) + q, :], 0.0)
                for wnum, amat in ((1, A_sb[X]), (2, A2_sb[X])):
                    for q in range(cfg.qt):
                        for b in range(BL):
                            pt = psum_e.tile([E, 128], F32, tag="pe", name="pt")
                            nc.tensor.transpose(
                                pt[:, :],
                                R_sb[:, cfg.qt * (3 * i + wnum) + q,
                                     E * b:E * (b + 1)],
                                ident[:, :])
                            xw = xwp.tile([E, 128], F32, tag="xw", name="xw")
                            nc.vector.tensor_copy(xw[:, :], pt[:, :])
                            ph = psum_m.tile([128, E], F32, tag="pr", name="ph")
                            nc.tensor.matmul(ph[:, :],
                                             lhsT=xw[:, :], rhs=amat[:, b, :],
                                             start=True, stop=True)
                            nc.vector.tensor_scalar_max(
                                hcat[:, cfg.qt * wnum + q, E * b:E * (b + 1)],
                                ph[:, :], 0.0)
                po = psum_m.tile([Z, N], F32, tag="pr")
                for k2 in range(cfg.hkt):
                    nc.tensor.matmul(po[:, :], lhsT=wout_sb[:, w_idx, k2, :],
                                     rhs=hcat[:, k2, :],
                                     start=(k2 == 0), stop=(k2 == cfg.hkt - 1))
                ot = opool.tile([Z, N], F32, tag="ot")
                nc.vector.tensor_scalar(ot[:, :], po[:, :],
                                        bout_sb[:, w_idx:w_idx + 1], None,
                                        op0=mybir.AluOpType.add)
                nc.sync.dma_start(zo.ap()[w_idx], ot[:, :])

        # ---- LN + reparam ----
        def layer_norm_reparam(slab_mu, slab_lv, nsz, g_rows, noise_row, out_off):
            """slab_*: [BL, nsz] tiles (modified in place)."""
            sub = _bn_split(nsz)
            g_mu, b_mu, g_lv, b_lv = g_rows

            def bcast(row):
                t = bc_p.tile([BL, nsz], F32, tag="bc", name="bc")
                src = ln_all.ap()[row:row + 1, 0:nsz]
                nc.sync.dma_start(
                    t[:, :], bass.AP(tensor=src.tensor, offset=src.offset,
                                     ap=[[0, BL], [1, nsz]]))
                return t

            for sl, g_row, b_row in ((slab_mu, g_mu, b_mu), (slab_lv, g_lv, b_lv)):
                st = slab_p.tile([BL, nsz // sub, 6], F32, tag="st", name="st")
                slview = sl.rearrange("p (a b) -> p a b", b=sub)
                for a in range(nsz // sub):
                    nc.vector.bn_stats(st[:, a, :], slview[:, a, :])
                mv2 = slab_p.tile([BL, 2], F32, tag="mv2", name="mv2")
                nc.vector.bn_aggr(mv2[:, :], st[:, :, :])
                sq = slab_p.tile([BL, 1], F32, tag="sq", name="sq")
                nc.scalar.activation(sq[:, :], mv2[:, 1:2], AF.Sqrt,
                                     bias=eps_sb[:, :])
                rstd = slab_p.tile([BL, 1], F32, tag="rstd", name="rstd")
                nc.vector.reciprocal(rstd[:, :], sq[:, :])
                # in-place: sl = ((sl - mean) * rstd) * g + b
                nc.vector.tensor_scalar(sl[:, :], sl[:, :], mv2[:, 0:1],
                                        rstd[:, :],
                                        op0=mybir.AluOpType.subtract,
                                        op1=mybir.AluOpType.mult)
                nc.vector.tensor_mul(sl[:, :], sl[:, :], bcast(g_row)[:, :])
                nc.vector.tensor_add(sl[:, :], sl[:, :], bcast(b_row)[:, :])
            # slab_lv = exp(0.5 * slab_lv); z = noise * that + slab_mu
            nc.scalar.activation(slab_lv[:, :], slab_lv[:, :], AF.Exp, scale=0.5)
            nz = slab_p.tile([BL, nsz], F32, tag="nz", name="nz")
            nc.sync.dma_start(nz[:, :], noise.ap()[noise_row:noise_row + BL, 0:nsz])
            nc.vector.tensor_mul(nz[:, :], nz[:, :], slab_lv[:, :])
            nc.vector.tensor_add(nz[:, :], nz[:, :], slab_mu[:, :])
            nc.sync.dma_start(out.ap()[:, out_off:out_off + nsz], nz[:, :])

        # vert: slabs [BL, V*Z] natural (v,z) order
        sv_mu = slab_p.tile([BL, cfg.se], F32, tag="smu", name="sv_mu")
        nc.sync.dma_start(sv_mu[:, 0:cfg.sv],
                          zv.ap().rearrange("j b v z -> j b (v z)")[0])
        sv_lv = slab_p.tile([BL, cfg.se], F32, tag="slv", name="sv_lv")
        nc.sync.dma_start(sv_lv[:, 0:cfg.sv],
                          zv.ap().rearrange("j b v z -> j b (v z)")[1])
        layer_norm_reparam(sv_mu[:, 0:cfg.sv], sv_lv[:, 0:cfg.sv], cfg.sv,
                           (8, 9, 10, 11), 8, 0)

        # edges: slabs [BL, Z*E] in (z,e) order from zo [w, Z, N]
        for gi, (w_mu, w_lv) in enumerate(((0, 1), (2, 3))):
            smu = slab_p.tile([BL, cfg.se], F32, tag="smu", name="smu")
            slv = slab_p.tile([BL, cfg.se], F32, tag="slv", name="slv")
            for w, sl in ((w_mu, smu), (w_lv, slv)):
                src = zo.ap()[w]  # [Z, N]; slab[b,(z,e)] = zo[w,z,48b+e]
                nc.sync.dma_start(
                    sl[:, :], bass.AP(tensor=src.tensor, offset=src.offset,
                                      ap=[[E, BL], [N, Z], [1, E]]))
            layer_norm_reparam(smu, slv, cfg.se, tuple(4 * gi + j for j in range(4)),
                               4 * gi, cfg.sv + cfg.se * gi)

    stack.close()


# ---------------- host side ----------------

def _pad(a, shape):
    r = np.zeros(shape, dtype=np.float32)
    r[tuple(slice(0, s) for s in a.shape)] = a
    return r


def prep_inputs(ops, inci_T, inci_H, noise_vert, noise_eT, noise_eH, params,
                cfg=FULL):
    p = params
    emb = np.asarray(p["emb"], np.float32)
    in_maps = []
    embT_h = _pad(emb.T, (cfg.kp, NT))
    wout_h = np.zeros((4, cfg.hcat, Z), np.float32)
    wcat = {}
    for w_idx, nm in enumerate(("muT", "lvT", "muH", "lvH")):
        pp = p[nm]
        for j, wn in enumerate(("W0", "W1", "W2")):
            wout_h[w_idx, cfg.hidp * j:cfg.hidp * j + cfg.hid, :] = \
                np.asarray(pp["Wout"], np.float32)[cfg.hid * j:cfg.hid * (j + 1), :]
    for X, names in (("T", ("muT", "lvT")), ("H", ("muH", "lvH"))):
        cat = np.zeros((cfg.dp, cfg.wcols), np.float32)
        for i, nm in enumerate(names):
            pp = p[nm]
            for j, wn in enumerate(("W0", "W1", "W2")):
                c0 = cfg.hidp * (3 * i + j)
                cat[:cfg.d, c0:c0 + cfg.hid] = np.asarray(pp[wn], np.float32)
        tiled = cat.reshape(cfg.kb, cfg.kk, 128, cfg.mt, 128)
        tiled = np.ascontiguousarray(tiled.transpose(3, 0, 2, 1, 4)).reshape(
            cfg.mt, cfg.kb, 128, cfg.kk * 128)
        hi = tiled.astype(ml_dtypes.bfloat16)
        lo = (tiled - hi.astype(np.float32)).astype(ml_dtypes.bfloat16)
        wcat[(X, "h")], wcat[(X, "l")] = hi, lo
    bout_h = np.stack([np.asarray(p[nm]["bout"], np.float32)
                       for nm in ("muT", "lvT", "muH", "lvH")])
    wg_hl = {}
    for X in "TH":
        wgp_ = _pad(np.asarray(p[f"Wg_{X}"], np.float32), (cfg.kp, cfg.dp))
        hi = wgp_.astype(ml_dtypes.bfloat16)
        lo = (wgp_ - hi.astype(np.float32)).astype(ml_dtypes.bfloat16)
        wg_hl[(X, "h")], wg_hl[(X, "l")] = hi, lo
    embTb_h = embT_h.astype(ml_dtypes.bfloat16)
    embTb_l = (embT_h - embTb_h.astype(np.float32)).astype(ml_dtypes.bfloat16)
    wv_h = {j: _pad(np.asarray(p[nm], np.float32), (cfg.kp, Z))
            for j, nm in ((0, "Wv_mu"), (1, "Wv_lv"))}
    bv_h = np.stack([np.asarray(p["bv_mu"], np.float32),
                     np.asarray(p["bv_lv"], np.float32)])
    ln = p["ln"]
    ln_h = np.zeros((12, cfg.se), np.float32)
    for gi, (gm, gl) in enumerate((("meT", "leT"), ("meH", "leH"))):
        ln_h[4 * gi + 0] = np.asarray(ln[gm]["g"], np.float32).T.ravel()
        ln_h[4 * gi + 1] = np.asarray(ln[gm]["b"], np.float32).T.ravel()
        ln_h[4 * gi + 2] = np.asarray(ln[gl]["g"], np.float32).T.ravel()
        ln_h[4 * gi + 3] = np.asarray(ln[gl]["b"], np.float32).T.ravel()
    ln_h[8, :cfg.sv] = np.asarray(ln["mv"]["g"], np.float32).ravel()
    ln_h[9, :cfg.sv] = np.asarray(ln["mv"]["b"], np.float32).ravel()
    ln_h[10, :cfg.sv] = np.asarray(ln["lv"]["g"], np.float32).ravel()
    ln_h[11, :cfg.sv] = np.asarray(ln["lv"]["b"], np.float32).ravel()

    ops_n = np.asarray(ops)
    onehot = np.eye(NT, dtype=np.float32)[ops_n]          # [B, V, NT]
    for c in range(NCORES):
        bs = slice(BL * c, BL * (c + 1))
        noise_h = np.zeros((12, cfg.se), np.float32)
        noise_h[0:4] = np.asarray(noise_eT, np.float32)[bs].transpose(0, 2, 1).reshape(BL, cfg.se)
        noise_h[4:8] = np.asarray(noise_eH, np.float32)[bs].transpose(0, 2, 1).reshape(BL, cfg.se)
        noise_h[8:12, :cfg.sv] = np.asarray(noise_vert, np.float32)[bs].reshape(BL, cfg.sv)
        m = {
            "embT": embT_h,
            "embTb_h": embTb_h, "embTb_l": embTb_l,
            "wg_T_h": np.ascontiguousarray(wg_hl[("T", "h")][:, cfg.sh * c:cfg.sh * (c + 1)]),
            "wg_T_l": np.ascontiguousarray(wg_hl[("T", "l")][:, cfg.sh * c:cfg.sh * (c + 1)]),
            "wg_H_h": np.ascontiguousarray(wg_hl[("H", "h")][:, cfg.sh * c:cfg.sh * (c + 1)]),
            "wg_H_l": np.ascontiguousarray(wg_hl[("H", "l")][:, cfg.sh * c:cfg.sh * (c + 1)]),
            "wcat_T_h": wcat[("T", "h")], "wcat_T_l": wcat[("T", "l")],
            "wcat_H_h": wcat[("H", "h")], "wcat_H_l": wcat[("H", "l")],
            "wout": wout_h, "bout": bout_h,
            "wv_0": wv_h[0], "wv_1": wv_h[1], "bv": bv_h,
            "inci_T": np.ascontiguousarray(np.asarray(inci_T, np.float32)[bs]),
            "inci_H": np.ascontiguousarray(np.asarray(inci_H, np.float32)[bs]),
            "onehot": np.ascontiguousarray(onehot[bs]),
            "onehotT": np.ascontiguousarray(onehot[bs].transpose(0, 2, 1)),
            "ln_all": ln_h, "noise": noise_h,
        }
        in_maps.append(m)
    return in_maps


def assemble_output(results, cfg=FULL):
    outs = []
    for c in range(NCORES):
        o = np.asarray(results[c]["out"], np.float32)     # [BL, sv+2*se]
        z_v = o[:, :cfg.sv].reshape(BL, V, Z)
        z_T = o[:, cfg.sv:cfg.sv + cfg.se].reshape(BL, Z, E).transpose(0, 2, 1)
        z_H = o[:, cfg.sv + cfg.se:].reshape(BL, Z, E).transpose(0, 2, 1)
        outs.append(np.concatenate([z_v, z_T, z_H], axis=1))
    return np.concatenate(outs, axis=0)


_STATE = {}


def _ensure_ntff_hook():
    """Inject antenv.axon_hooks (missing in this image) so trace=True works."""
    import sys, types
    try:
        import antenv.axon_hooks  # noqa: F401
        return
    except ImportError:
        pass
    try:
        from trn_agent_boot.trn_boot import _ntff_profile_via_ctypes
        hook = _ntff_profile_via_ctypes("/opt/axon/libaxon_pjrt.so")
        mod = types.ModuleType("antenv.axon_hooks")
        mod._hook = hook
        mod.set_axon_ntff_profile_hook = lambda h: setattr(mod, "_hook", h)
        mod.get_axon_ntff_profile_hook = lambda: mod._hook
        sys.modules["antenv.axon_hooks"] = mod
        import antenv
        antenv.axon_hooks = mod
    except Exception:
        pass


def kernel(ops, inci_T, inci_H, noise_vert, noise_eT, noise_eH, params):
    from concourse.bass_utils import run_bass_kernel_spmd
    if bool(int(os.environ.get("BASS_KERNEL_TRACE", "0"))):
        _ensure_ntff_hook()
    if "nc" not in _STATE:
        _STATE["nc"] = build_nc(FULL)
    nc = _STATE["nc"]
    in_maps = prep_inputs(ops, inci_T, inci_H, noise_vert, noise_eT, noise_eH,
                          params, FULL)
    trace = bool(int(os.environ.get("BASS_KERNEL_TRACE", "0")))
    res = run_bass_kernel_spmd(nc, in_maps, core_ids=list(range(NCORES)),
                               trace=trace)
    _STATE["last_result"] = res
    return assemble_output(res.results, FULL)


# revision 13
# speedup vs baseline: 1.7372x; 1.6789x over previous
"""Trainium2 Bass kernel for DVAE_INCI_WO_GNN forward (8 NeuronCores, SPMD).

Strategy (batch-parallel, 4 batches/core):
  - P = emb @ Wg computed on device with Wg column-sharded 8 ways (each core
    reads 1/8 of the two 10020x10020 matrices), then AllGather of the tiny
    [10, D] P matrices.  (hidden @ Wg == (emb @ Wg)[ops] since hidden rows
    are drawn from the 10-row embedding table.)
  - edges^T[b] = P^T @ M[b]^T with M[b] = onehot(ops[b])^T @ inci[b]  (rank-10).
  - inception: h1 = relu(A @ (x@W1)), h2 = relu(A^2 @ (x@W2)), A = inci^T inci.
    Main matmul (x @ [W0|W1|W2] for 4 inceptions) is batch-sharded; weights are
    read once per core as host-split bf16 hi/lo pairs and multiplied in a
    3-pass split-bf16 scheme (fp32-grade accuracy at 3/4 of fp32 PE cost).
  - LayerNorm over (rows, Z) slabs + uniform-noise reparam done per-core on
    [4, slab] layouts, output gathered on host.
"""

import os
from contextlib import ExitStack
import numpy as np
import ml_dtypes

import concourse.bass as bass
import concourse.tile as tile
from concourse import bacc, mybir
from concourse.masks import make_identity

F32 = mybir.dt.float32
BF16 = mybir.dt.bfloat16
AF = mybir.ActivationFunctionType

# Precision mode: 1 = single-pass bf16 (fast, ~3e-3 rel err);
# 3 = split-bf16 3-pass (fp32-grade, ~5e-6 rel err), and fp32 edges/P AG.
PASSES = 1

NCORES = 8
B, V, E, NT, Z = 32, 24, 48, 10, 56
BL = B // NCORES          # batches per core = 4
N = BL * E                # main matmul moving dim = 192
D = 10020
HID = 501


class Cfg:
    def __init__(self, d=D, hid=HID, name="full"):
        self.name = name
        self.d = d
        self.hid = hid
        self.dp = ((d + 1023) // 1024) * 1024      # D padded: shard*8, shard%128==0
        self.sh = self.dp // NCORES                # cols per core
        self.nch = self.sh // 128                  # P chunks per core
        self.kp = ((d + 127) // 128) * 128         # Wg row pad (P contraction)
        self.kt_p = self.kp // 128                 # P ktiles
        self.kt = self.dp // 128                   # main-matmul ktiles
        self.kb = 4                                # k blocks in main loop
        assert self.kt % self.kb == 0
        self.kk = self.kt // self.kb               # ktiles per block
        self.hidp = ((hid + 127) // 128) * 128     # padded W block width
        self.qt = self.hidp // 128                 # R Mtiles per W block
        self.wcols = 6 * self.hidp                 # Wcat cols per matrix
        self.mt = self.wcols // 128                # Wcat Mtiles per matrix
        self.hcat = 3 * self.hidp                  # padded concat width
        self.hkt = self.hcat // 128                # Wout ktiles
        self.se = Z * E                            # edge slab size = 2688
        self.sv = V * Z                            # vert slab size = 1344


FULL = Cfg()


def _bn_split(n):
    """free-dim subgroup size <= 512 dividing n."""
    for s in (512, 448, 384, 336, 256, 224, 192, 128, 112, 64, 56, 48, 32, 16, 8, 7, 4, 2, 1):
        if n % s == 0 and s <= 512:
            return s
    return 1


def build_nc(cfg=FULL):
    nc = bacc.Bacc("TRN2", target_bir_lowering=False, debug=False,
                   num_devices=NCORES)

    dI = lambda name, shape, dt=F32: nc.dram_tensor(name, shape, dt, kind="ExternalInput")
    # ---- parameters (per-core shards / replicas) ----
    embT = dI("embT", [cfg.kp, NT])                       # emb^T padded
    hs = "hl" if PASSES == 3 else "h"
    wg = {(X, h): dI(f"wg_{X}_{h}", [cfg.kp, cfg.sh], BF16)
          for X in "TH" for h in hs}
    embTb = {h: dI(f"embTb_{h}", [cfg.kp, NT], BF16) for h in hs}
    wc = {(X, h): dI(f"wcat_{X}_{h}", [cfg.mt, cfg.kb, 128, cfg.kk * 128], BF16)
          for X in "TH" for h in hs}
    wout = dI("wout", [4, cfg.hcat, Z])                   # muT,lvT,muH,lvH
    bout = dI("bout", [4, Z])
    wv = {j: dI(f"wv_{j}", [cfg.kp, Z]) for j in (0, 1)}  # mu, lv
    bv = dI("bv", [2, Z])
    inci = {X: dI(f"inci_{X}", [BL, V, E]) for X in "TH"}
    oneh = dI("onehot", [BL, V, NT])
    onehT = dI("onehotT", [BL, NT, V])
    ln_all = dI("ln_all", [12, cfg.se])                   # g/b packed rows
    noise = dI("noise", [12, cfg.se])                     # eT b0-3, eH b0-3, vert b0-3
    out = nc.dram_tensor("out", [BL, cfg.sv + 2 * cfg.se], F32, kind="ExternalOutput")

    # ---- internal DRAM ----
    AGDT = F32 if PASSES == 3 else BF16
    ag_in = nc.dram_tensor("ag_in", [2 * NT, cfg.sh], AGDT)
    ag_out = nc.dram_tensor("ag_out", [NCORES * 2 * NT, cfg.sh], AGDT)
    zo = nc.dram_tensor("zo", [4, Z, N], F32)             # (X,i) out_incep^T scratch
    zv = nc.dram_tensor("zv", [2, BL, V, Z], F32)         # vert mu/lv scratch

    with tile.TileContext(nc) as tc:
        _emit(tc, cfg, embT, embTb, wg, wc, wout, bout, wv, bv, inci, oneh, onehT,
              ln_all, noise, out, ag_in, ag_out, zo, zv)
    nc.compile()
    return nc


def _emit(tc, cfg, embT, embTb, wg, wc, wout, bout, wv, bv, inci, oneh, onehT,
          ln_all, noise, out, ag_in, ag_out, zo, zv):
    nc = tc.nc
    TH = "TH"

    stack = ExitStack()
    const = stack.enter_context(tc.tile_pool(name="const", bufs=1))
    psmall = stack.enter_context(tc.tile_pool(name="psmall", bufs=2, space="PSUM"))

    # ---- constants into SBUF ----
    embT_sb = const.tile([128, cfg.kt_p, NT], F32)
    nc.sync.dma_start(embT_sb[:, :, :],
                      embT.ap().rearrange("(k p) t -> p k t", p=128))
    embTb_sb = {}
    for h in ("hl" if PASSES == 3 else "h"):
        embTb_sb[h] = const.tile([128, cfg.kt_p, NT], BF16, tag=f"embTb{h}",
                                 name=f"embTb_sb{h}")
        nc.sync.dma_start(embTb_sb[h][:, :, :],
                          embTb[h].ap().rearrange("(k p) t -> p k t", p=128))
    ident = const.tile([128, 128], F32)
    make_identity(nc, ident[:, :])
    inci_sb = {}
    for X in TH:
        inci_sb[X] = const.tile([V, BL, E], F32, tag=f"inci{X}", name=f"inci_sb{X}")
        nc.sync.dma_start(inci_sb[X][:, :, :],
                          inci[X].ap().rearrange("b v e -> v b e"))
    oneh_sb = const.tile([V, BL, NT], F32)
    nc.sync.dma_start(oneh_sb[:, :, :], oneh.ap().rearrange("b v t -> v b t"))
    onehT_sb = const.tile([NT, BL, V], F32)
    nc.sync.dma_start(onehT_sb[:, :, :], onehT.ap().rearrange("b t v -> t b v"))
    wout_sb = const.tile([128, 4, cfg.hkt, Z], F32)
    nc.sync.dma_start(wout_sb[:, :, :, :],
                      wout.ap().rearrange("w (k p) z -> p w k z", p=128))
    bout_sb = const.tile([Z, 4], F32)
    nc.sync.dma_start(bout_sb[:, :], bout.ap().rearrange("w z -> z w"))
    eps_sb = const.tile([BL, 1], F32)
    nc.vector.memset(eps_sb[:, :], 1e-5)

    # ---- A, A2, M (small PE work, runs during Wg streaming) ----
    A_sb, A2_sb, M_sb = {}, {}, {}
    for X in TH:
        A_sb[X] = const.tile([E, BL, E], F32, tag=f"A{X}", name=f"A_sb{X}")
        A2_sb[X] = const.tile([E, BL, E], F32, tag=f"A2{X}", name=f"A2_sb{X}")
        MDT = F32 if PASSES == 3 else BF16
        M_sb[X] = const.tile([NT, N], MDT, tag=f"M{X}", name=f"M_sb{X}")
        for b in range(BL):
            pm = psmall.tile([NT, E], F32, tag="ps", name="pm")
            nc.tensor.matmul(pm[:, :],
                             lhsT=oneh_sb[:, b, :], rhs=inci_sb[X][:, b, :],
                             start=True, stop=True)
            nc.vector.tensor_copy(M_sb[X][:, E * b:E * (b + 1)], pm[:, :])
        for b in range(BL):
            pa = psmall.tile([E, E], F32, tag="ps")
            nc.tensor.matmul(pa[:, :], lhsT=inci_sb[X][:, b, :],
                             rhs=inci_sb[X][:, b, :], start=True, stop=True)
            nc.vector.tensor_copy(A_sb[X][:, b, :], pa[:, :])
            pa2 = psmall.tile([E, E], F32, tag="ps")
            nc.tensor.matmul(pa2[:, :], lhsT=A_sb[X][:, b, :],
                             rhs=A_sb[X][:, b, :], start=True, stop=True)
            nc.vector.tensor_copy(A2_sb[X][:, b, :], pa2[:, :])

    # ---- Phase P: P = emb @ Wg (column shard), AllGather ----
    with tc.tile_pool(name="wgp", bufs=3) as wgp, \
         tc.tile_pool(name="psum_p", bufs=1, space="PSUM") as psum_p:
        psP = {X: psum_p.tile([NT, cfg.sh], F32, tag=f"psP{X}", name=f"psP{X}") for X in TH}
        for k in range(cfg.kt_p):
            for X in TH:
                wgh = wgp.tile([128, cfg.sh], BF16, tag="wgh", name="wgh")
                nc.sync.dma_start(wgh[:, :], wg[(X, "h")][128 * k:128 * (k + 1), :])
                if PASSES == 3:
                    wgl = wgp.tile([128, cfg.sh], BF16, tag="wgl", name="wgl")
                    nc.sync.dma_start(wgl[:, :],
                                      wg[(X, "l")][128 * k:128 * (k + 1), :])
                    passes = ((embTb_sb["h"], wgh), (embTb_sb["h"], wgl),
                              (embTb_sb["l"], wgh))
                else:
                    passes = ((embTb_sb["h"], wgh),)
                na = len(passes)
                for a, (lh, rh) in enumerate(passes):
                    for c0 in range(0, cfg.sh, 512):
                        c1 = min(c0 + 512, cfg.sh)
                        nc.tensor.matmul(psP[X][:, c0:c1],
                                         lhsT=lh[:, k, :],
                                         rhs=rh[:, c0:c1],
                                         start=(k == 0 and a == 0),
                                         stop=(k == cfg.kt_p - 1 and a == na - 1))
        for xi, X in enumerate(TH):
            stg = wgp.tile([NT, cfg.sh], F32 if PASSES == 3 else BF16, tag="stg")
            nc.vector.tensor_copy(stg[:, :], psP[X][:, :])
            nc.sync.dma_start(ag_in[NT * xi:NT * (xi + 1), :], stg[:, :])
    nc.gpsimd.collective_compute(
        "AllGather", mybir.AluOpType.bypass,
        replica_groups=[list(range(NCORES))],
        ins=[ag_in.ap().opt()], outs=[ag_out.ap().opt()])

    # ---- vert path: Pv = emb @ Wv, q = relu(Pv + bv), gather by ops ----
    with tc.tile_pool(name="wvp", bufs=3) as wvp:
        q_sb = {}
        for j in (0, 1):
            psq = psmall.tile([NT, Z], F32, tag="ps")
            for k in range(cfg.kt_p):
                wvt = wvp.tile([128, Z], F32, tag="wv")
                nc.sync.dma_start(wvt[:, :], wv[j][128 * k:128 * (k + 1), :])
                nc.tensor.matmul(psq[:, :], lhsT=embT_sb[:, k, :], rhs=wvt[:, :],
                                 start=(k == 0), stop=(k == cfg.kt_p - 1))
            bvb = wvp.tile([NT, Z], F32, tag="bvb")
            src = bv.ap()[j:j + 1, :]
            nc.sync.dma_start(bvb[:, :],
                              bass.AP(tensor=src.tensor, offset=src.offset,
                                      ap=[[0, NT], [1, Z]]))
            tq = wvp.tile([NT, Z], F32, tag="tq")
            nc.vector.tensor_add(tq[:, :], psq[:, :], bvb[:, :])
            q_sb[j] = const.tile([NT, Z], F32, tag=f"q{j}", name=f"q_sb{j}")
            nc.vector.tensor_scalar_max(q_sb[j][:, :], tq[:, :], 0.0)
            vq = wvp.tile([V, BL, Z], F32, tag="vq")
            for b in range(BL):
                pg = psmall.tile([V, Z], F32, tag="ps")
                nc.tensor.matmul(pg[:, :], lhsT=onehT_sb[:, b, :], rhs=q_sb[j][:, :],
                                 start=True, stop=True)
                nc.vector.tensor_copy(vq[:, b, :], pg[:, :])
            for b in range(BL):
                nc.sync.dma_start(zv.ap()[j, b, :, :], vq[:, b, :])

    # ---- main per-matrix pipeline ----
    with tc.tile_pool(name="plhs", bufs=4) as plhs_p, \
         tc.tile_pool(name="epool", bufs=1) as epool, \
         tc.tile_pool(name="etmp", bufs=2) as etmp, \
         tc.tile_pool(name="wcp", bufs=3) as wcp, \
         tc.tile_pool(name="rpool", bufs=1) as rpool, \
         tc.tile_pool(name="hpool", bufs=1) as hpool, \
         tc.tile_pool(name="xwp", bufs=4) as xwp, \
         tc.tile_pool(name="opool", bufs=2) as opool, \
         tc.tile_pool(name="slab", bufs=1) as slab_p, \
         tc.tile_pool(name="bcast", bufs=1) as bc_p, \
         tc.tile_pool(name="psum_e", bufs=2, space="PSUM") as psum_e, \
         tc.tile_pool(name="psum_m", bufs=2, space="PSUM") as psum_m:

        for xi, X in enumerate(TH):
            # -- edges: e^T [dp, N] in bf16 hi/lo --
            e_hi = epool.tile([128, cfg.kt, N], BF16, tag="ehi")
            e_lo = (epool.tile([128, cfg.kt, N], BF16, tag="elo", name="e_lo")
                    if PASSES == 3 else None)
            for k in range(cfg.kt):
                r, kk = divmod(k, cfg.nch)
                pl = plhs_p.tile([NT, 128], F32 if PASSES == 3 else BF16, tag="pl")
                nc.sync.dma_start(
                    pl[:, :],
                    ag_out.ap()[2 * NT * r + NT * xi: 2 * NT * r + NT * (xi + 1),
                                128 * kk:128 * (kk + 1)])
                pe = psum_e.tile([128, N], F32, tag="pe")
                nc.tensor.matmul(pe[:, :], lhsT=pl[:, :], rhs=M_sb[X][:, :],
                                 start=True, stop=True)
                nc.scalar.activation(e_hi[:, k, :], pe[:, :], AF.Relu)
                if PASSES == 3:
                    tmp = etmp.tile([128, N], F32, tag="etmp")
                    nc.scalar.activation(tmp[:, :], pe[:, :], AF.Relu)
                    hi32 = etmp.tile([128, N], F32, tag="hi32")
                    nc.vector.tensor_copy(hi32[:, :], e_hi[:, k, :])
                    diff = etmp.tile([128, N], F32, tag="diff")
                    nc.vector.tensor_sub(diff[:, :], tmp[:, :], hi32[:, :])
                    nc.vector.tensor_copy(e_lo[:, k, :], diff[:, :])

            # -- main matmul: R = (x @ Wcat)^T, 3-pass split bf16 --
            R_sb = rpool.tile([128, cfg.mt, N], F32, tag="R")
            for m in range(cfg.mt):
                pr = psum_m.tile([128, N], F32, tag="pr")
                for kb in range(cfg.kb):
                    sl = {}
                    for h in ("hl" if PASSES == 3 else "h"):
                        wt = wcp.tile([128, cfg.kk, 128], BF16, tag=f"wc{h}")
                        nc.sync.dma_start(
                            wt[:, :, :],
                            wc[(X, h)].ap()[m, kb].rearrange(
                                "p (kk c) -> p kk c", c=128))
                        sl[h] = wt
                    for kk in range(cfg.kk):
                        k = cfg.kk * kb + kk
                        last = (k == cfg.kt - 1)
                        if PASSES == 3:
                            nc.tensor.matmul(pr[:, :], lhsT=sl["h"][:, kk, :],
                                             rhs=e_hi[:, k, :],
                                             start=(k == 0), stop=False)
                            nc.tensor.matmul(pr[:, :], lhsT=sl["h"][:, kk, :],
                                             rhs=e_lo[:, k, :],
                                             start=False, stop=False)
                            nc.tensor.matmul(pr[:, :], lhsT=sl["l"][:, kk, :],
                                             rhs=e_hi[:, k, :],
                                             start=False, stop=last)
                        else:
                            nc.tensor.matmul(pr[:, :], lhsT=sl["h"][:, kk, :],
                                             rhs=e_hi[:, k, :],
                                             start=(k == 0), stop=last)
                nc.vector.tensor_copy(R_sb[:, m, :], pr[:, :])

            # -- final per inception i: hcat^T -> Wout -> zo --
            for i in range(2):
                w_idx = 2 * xi + i
                hcat = hpool.tile([128, cfg.hkt, N], F32, tag="hcat")
                for q in range(cfg.qt):
                    nc.vector.tensor_scalar_max(
                        hcat[:, q, :], R_sb[:, cfg.qt * (3 * i) + q, :], 0.0)
                for wnum, amat in ((1, A_sb[X]), (2, A2_sb[X])):
                    for q in range(cfg.qt):
                        for b in range(BL):
                            pt = psum_e.tile([E, 128], F32, tag="pe", name="pt")
                            nc.tensor.transpose(
                                pt[:, :],
                                R_sb[:, cfg.qt * (3 * i + wnum) + q,
                                     E * b:E * (b + 1)],
                                ident[:, :])
                            xw = xwp.tile([E, 128], F32, tag="xw", name="xw")
                            nc.vector.tensor_copy(xw[:, :], pt[:, :])
                            ph = psum_m.tile([128, E], F32, tag="pr", name="ph")
                            nc.tensor.matmul(ph[:, :],
                                             lhsT=xw[:, :], rhs=amat[:, b, :],
                                             start=True, stop=True)
                            nc.vector.tensor_scalar_max(
                                hcat[:, cfg.qt * wnum + q, E * b:E * (b + 1)],
                                ph[:, :], 0.0)
                po = psum_m.tile([Z, N], F32, tag="pr")
                for k2 in range(cfg.hkt):
                    nc.tensor.matmul(po[:, :], lhsT=wout_sb[:, w_idx, k2, :],
                                     rhs=hcat[:, k2, :],
                                     start=(k2 == 0), stop=(k2 == cfg.hkt - 1))
                ot = opool.tile([Z, N], F32, tag="ot")
                nc.vector.tensor_scalar(ot[:, :], po[:, :],
                                        bout_sb[:, w_idx:w_idx + 1], None,
                                        op0=mybir.AluOpType.add)
                nc.sync.dma_start(zo.ap()[w_idx], ot[:, :])

        # ---- LN + reparam ----
        def layer_norm_reparam(slab_mu, slab_lv, nsz, g_rows, noise_row, out_off):
            """slab_*: [BL, nsz] tiles (modified in place)."""
            sub = _bn_split(nsz)
            g_mu, b_mu, g_lv, b_lv = g_rows

            def bcast(row):
                t = bc_p.tile([BL, nsz], F32, tag="bc", name="bc")
                src = ln_all.ap()[row:row + 1, 0:nsz]
                nc.sync.dma_start(
                    t[:, :], bass.AP(tensor=src.tensor, offset=src.offset,
                                     ap=[[0, BL], [1, nsz]]))
                return t

            for sl, g_row, b_row in ((slab_mu, g_mu, b_mu), (slab_lv, g_lv, b_lv)):
                st = slab_p.tile([BL, nsz // sub, 6], F32, tag="st", name="st")
                slview = sl.rearrange("p (a b) -> p a b", b=sub)
                for a in range(nsz // sub):
                    nc.vector.bn_stats(st[:, a, :], slview[:, a, :])
                mv2 = slab_p.tile([BL, 2], F32, tag="mv2", name="mv2")
                nc.vector.bn_aggr(mv2[:, :], st[:, :, :])
                sq = slab_p.tile([BL, 1], F32, tag="sq", name="sq")
                nc.scalar.activation(sq[:, :], mv2[:, 1:2], AF.Sqrt,
                                     bias=eps_sb[:, :])
                rstd = slab_p.tile([BL, 1], F32, tag="rstd", name="rstd")
                nc.vector.reciprocal(rstd[:, :], sq[:, :])
                # in-place: sl = ((sl - mean) * rstd) * g + b
                nc.vector.tensor_scalar(sl[:, :], sl[:, :], mv2[:, 0:1],
                                        rstd[:, :],
                                        op0=mybir.AluOpType.subtract,
                                        op1=mybir.AluOpType.mult)
                nc.vector.tensor_mul(sl[:, :], sl[:, :], bcast(g_row)[:, :])
                nc.vector.tensor_add(sl[:, :], sl[:, :], bcast(b_row)[:, :])
            # slab_lv = exp(0.5 * slab_lv); z = noise * that + slab_mu
            nc.scalar.activation(slab_lv[:, :], slab_lv[:, :], AF.Exp, scale=0.5)
            nz = slab_p.tile([BL, nsz], F32, tag="nz", name="nz")
            nc.sync.dma_start(nz[:, :], noise.ap()[noise_row:noise_row + BL, 0:nsz])
            nc.vector.tensor_mul(nz[:, :], nz[:, :], slab_lv[:, :])
            nc.vector.tensor_add(nz[:, :], nz[:, :], slab_mu[:, :])
            nc.sync.dma_start(out.ap()[:, out_off:out_off + nsz], nz[:, :])

        # vert: slabs [BL, V*Z] natural (v,z) order
        sv_mu = slab_p.tile([BL, cfg.se], F32, tag="smu", name="sv_mu")
        nc.sync.dma_start(sv_mu[:, 0:cfg.sv],
                          zv.ap().rearrange("j b v z -> j b (v z)")[0])
        sv_lv = slab_p.tile([BL, cfg.se], F32, tag="slv", name="sv_lv")
        nc.sync.dma_start(sv_lv[:, 0:cfg.sv],
                          zv.ap().rearrange("j b v z -> j b (v z)")[1])
        layer_norm_reparam(sv_mu[:, 0:cfg.sv], sv_lv[:, 0:cfg.sv], cfg.sv,
                           (8, 9, 10, 11), 8, 0)

        # edges: slabs [BL, Z*E] in (z,e) order from zo [w, Z, N]
        for gi, (w_mu, w_lv) in enumerate(((0, 1), (2, 3))):
            smu = slab_p.tile([BL, cfg.se], F32, tag="smu", name="smu")
            slv = slab_p.tile([BL, cfg.se], F32, tag="slv", name="slv")
            for w, sl in ((w_mu, smu), (w_lv, slv)):
                src = zo.ap()[w]  # [Z, N]; slab[b,(z,e)] = zo[w,z,48b+e]
                nc.sync.dma_start(
                    sl[:, :], bass.AP(tensor=src.tensor, offset=src.offset,
                                      ap=[[E, BL], [N, Z], [1, E]]))
            layer_norm_reparam(smu, slv, cfg.se, tuple(4 * gi + j for j in range(4)),
                               4 * gi, cfg.sv + cfg.se * gi)

    stack.close()


# ---------------- host side ----------------

def _pad(a, shape):
    r = np.zeros(shape, dtype=np.float32)
    r[tuple(slice(0, s) for s in a.shape)] = a
    return r


def prep_inputs(ops, inci_T, inci_H, noise_vert, noise_eT, noise_eH, params,
                cfg=FULL):
    p = params
    emb = np.asarray(p["emb"], np.float32)
    in_maps = []
    embT_h = _pad(emb.T, (cfg.kp, NT))
    wout_h = np.zeros((4, cfg.hcat, Z), np.float32)
    wcat = {}
    for w_idx, nm in enumerate(("muT", "lvT", "muH", "lvH")):
        pp = p[nm]
        for j, wn in enumerate(("W0", "W1", "W2")):
            wout_h[w_idx, cfg.hidp * j:cfg.hidp * j + cfg.hid, :] = \
                np.asarray(pp["Wout"], np.float32)[cfg.hid * j:cfg.hid * (j + 1), :]
    for X, names in (("T", ("muT", "lvT")), ("H", ("muH", "lvH"))):
        cat = np.zeros((cfg.dp, cfg.wcols), np.float32)
        for i, nm in enumerate(names):
            pp = p[nm]
            for j, wn in enumerate(("W0", "W1", "W2")):
                c0 = cfg.hidp * (3 * i + j)
                cat[:cfg.d, c0:c0 + cfg.hid] = np.asarray(pp[wn], np.float32)
        tiled = cat.reshape(cfg.kb, cfg.kk, 128, cfg.mt, 128)
        tiled = np.ascontiguousarray(tiled.transpose(3, 0, 2, 1, 4)).reshape(
            cfg.mt, cfg.kb, 128, cfg.kk * 128)
        hi = tiled.astype(ml_dtypes.bfloat16)
        wcat[(X, "h")] = hi
        if PASSES == 3:
            wcat[(X, "l")] = (tiled - hi.astype(np.float32)).astype(ml_dtypes.bfloat16)
    bout_h = np.stack([np.asarray(p[nm]["bout"], np.float32)
                       for nm in ("muT", "lvT", "muH", "lvH")])
    wg_hl = {}
    for X in "TH":
        wgp_ = _pad(np.asarray(p[f"Wg_{X}"], np.float32), (cfg.kp, cfg.dp))
        hi = wgp_.astype(ml_dtypes.bfloat16)
        wg_hl[(X, "h")] = hi
        if PASSES == 3:
            wg_hl[(X, "l")] = (wgp_ - hi.astype(np.float32)).astype(ml_dtypes.bfloat16)
    embTb_h = embT_h.astype(ml_dtypes.bfloat16)
    embTb_l = (embT_h - embTb_h.astype(np.float32)).astype(ml_dtypes.bfloat16)
    wv_h = {j: _pad(np.asarray(p[nm], np.float32), (cfg.kp, Z))
            for j, nm in ((0, "Wv_mu"), (1, "Wv_lv"))}
    bv_h = np.stack([np.asarray(p["bv_mu"], np.float32),
                     np.asarray(p["bv_lv"], np.float32)])
    ln = p["ln"]
    ln_h = np.zeros((12, cfg.se), np.float32)
    for gi, (gm, gl) in enumerate((("meT", "leT"), ("meH", "leH"))):
        ln_h[4 * gi + 0] = np.asarray(ln[gm]["g"], np.float32).T.ravel()
        ln_h[4 * gi + 1] = np.asarray(ln[gm]["b"], np.float32).T.ravel()
        ln_h[4 * gi + 2] = np.asarray(ln[gl]["g"], np.float32).T.ravel()
        ln_h[4 * gi + 3] = np.asarray(ln[gl]["b"], np.float32).T.ravel()
    ln_h[8, :cfg.sv] = np.asarray(ln["mv"]["g"], np.float32).ravel()
    ln_h[9, :cfg.sv] = np.asarray(ln["mv"]["b"], np.float32).ravel()
    ln_h[10, :cfg.sv] = np.asarray(ln["lv"]["g"], np.float32).ravel()
    ln_h[11, :cfg.sv] = np.asarray(ln["lv"]["b"], np.float32).ravel()

    ops_n = np.asarray(ops)
    onehot = np.eye(NT, dtype=np.float32)[ops_n]          # [B, V, NT]
    for c in range(NCORES):
        bs = slice(BL * c, BL * (c + 1))
        noise_h = np.zeros((12, cfg.se), np.float32)
        noise_h[0:4] = np.asarray(noise_eT, np.float32)[bs].transpose(0, 2, 1).reshape(BL, cfg.se)
        noise_h[4:8] = np.asarray(noise_eH, np.float32)[bs].transpose(0, 2, 1).reshape(BL, cfg.se)
        noise_h[8:12, :cfg.sv] = np.asarray(noise_vert, np.float32)[bs].reshape(BL, cfg.sv)
        m = {
            "embT": embT_h,
            "wout": wout_h, "bout": bout_h,
            "wv_0": wv_h[0], "wv_1": wv_h[1], "bv": bv_h,
            "inci_T": np.ascontiguousarray(np.asarray(inci_T, np.float32)[bs]),
            "inci_H": np.ascontiguousarray(np.asarray(inci_H, np.float32)[bs]),
            "onehot": np.ascontiguousarray(onehot[bs]),
            "onehotT": np.ascontiguousarray(onehot[bs].transpose(0, 2, 1)),
            "ln_all": ln_h, "noise": noise_h,
        }
        m["embTb_h"] = embTb_h
        if PASSES == 3:
            m["embTb_l"] = embTb_l
        for X in "TH":
            for h in ("hl" if PASSES == 3 else "h"):
                m[f"wg_{X}_{h}"] = np.ascontiguousarray(
                    wg_hl[(X, h)][:, cfg.sh * c:cfg.sh * (c + 1)])
                m[f"wcat_{X}_{h}"] = wcat[(X, h)]
        in_maps.append(m)
    return in_maps


def assemble_output(results, cfg=FULL):
    outs = []
    for c in range(NCORES):
        o = np.asarray(results[c]["out"], np.float32)     # [BL, sv+2*se]
        z_v = o[:, :cfg.sv].reshape(BL, V, Z)
        z_T = o[:, cfg.sv:cfg.sv + cfg.se].reshape(BL, Z, E).transpose(0, 2, 1)
        z_H = o[:, cfg.sv + cfg.se:].reshape(BL, Z, E).transpose(0, 2, 1)
        outs.append(np.concatenate([z_v, z_T, z_H], axis=1))
    return np.concatenate(outs, axis=0)


_STATE = {}


def _ensure_ntff_hook():
    """Inject antenv.axon_hooks (missing in this image) so trace=True works."""
    import sys, types
    try:
        import antenv.axon_hooks  # noqa: F401
        return
    except ImportError:
        pass
    try:
        from trn_agent_boot.trn_boot import _ntff_profile_via_ctypes
        hook = _ntff_profile_via_ctypes("/opt/axon/libaxon_pjrt.so")
        mod = types.ModuleType("antenv.axon_hooks")
        mod._hook = hook
        mod.set_axon_ntff_profile_hook = lambda h: setattr(mod, "_hook", h)
        mod.get_axon_ntff_profile_hook = lambda: mod._hook
        sys.modules["antenv.axon_hooks"] = mod
        import antenv
        antenv.axon_hooks = mod
    except Exception:
        pass


def kernel(ops, inci_T, inci_H, noise_vert, noise_eT, noise_eH, params):
    from concourse.bass_utils import run_bass_kernel_spmd
    if bool(int(os.environ.get("BASS_KERNEL_TRACE", "0"))):
        _ensure_ntff_hook()
    if "nc" not in _STATE:
        _STATE["nc"] = build_nc(FULL)
    nc = _STATE["nc"]
    in_maps = prep_inputs(ops, inci_T, inci_H, noise_vert, noise_eT, noise_eH,
                          params, FULL)
    trace = bool(int(os.environ.get("BASS_KERNEL_TRACE", "0")))
    res = run_bass_kernel_spmd(nc, in_maps, core_ids=list(range(NCORES)),
                               trace=trace)
    _STATE["last_result"] = res
    return assemble_output(res.results, FULL)
